# revision 23
# baseline (speedup 1.0000x reference)
"""Single-head causal attention on 8 Trainium2 NeuronCores.

Problem: x[B=8, T=2048, E=1024] fp32, Wq/Wk/Wv [E, H=64] fp32.
    q = x @ Wq; k = x @ Wk; v = x @ Wv
    out = softmax(causal(q @ k^T / sqrt(H))) @ v          -> [8, 2048, 64]

Sharding: pure data parallel, one batch element per core; weights replicated.

Per-core kernel design (transposed-scores formulation):
  - xT[e, t] built from x via PE transpose-mode (fp32, exact) - every
    projection contracts over e, so e must be on the partition axis.
  - qT/kT [64, 2048] via lhsT=W-chunk [128e, 64], rhs=xT [128e, 512t].
  - vT [64, 2048] likewise, then 16 small PE transposes -> vaug[s-chunk] =
    [v | 1] tiles [128, 65] (ones column makes the softmax denominator fall
    out of the AV matmul for free).
  - scoresT[s, t] = kT_j.T @ qT (PSUM), exp(0.125*scores) on ACT -> SBUF.
    Causal: only blocks j<=t-chunk are computed; diagonal blocks get a
    triangular 0/1 multiplicative mask; sub-diagonal column spans zeroed.
    exp without max-subtraction is safe: |scores| <~ 6 so exp in [e-6, e6].
  - outT[65, 512] accumulates vaug_j.T @ expT_j over j; row 64 = denominator.
    Small PE transpose back to [t, 65], divide rows by denominator, DMA out.
  - Big matmuls (proj/scores/AV) can run as float32r (full-rate fp32
    streaming) or float32 (4 cycles/row, bit-accurate); transposes always
    run in fp32 transpose-mode which is exact data movement.
"""

import os

import numpy as np

import concourse.bacc as bacc
import concourse.bass as bass
import concourse.tile as tile
from concourse import mybir
from concourse.bass_utils import run_bass_kernel_spmd
from concourse.masks import make_identity

B, T, E, H = 8, 2048, 1024, 64
P = 128                      # SBUF partitions
NE = E // P                  # 8 e-chunks
NT = T // P                  # 16 t-chunks (also s-chunks)
GW = 512                     # t-group width (matmul moving-operand max, fp32)
NG = T // GW                 # 4 t-groups
CPG = GW // P                # 4 chunks per group
F32 = mybir.dt.float32

# Matmul dtype for the heavy matmuls: "f32r" (fast) or "f32" (exact).
MM_DTYPE = os.environ.get("ATTN_MM_DTYPE", "f32r")

_NC_CACHE: dict = {}




def build_attention_nc(mm_dtype: str = "f32r", repeat: int = 1) -> bass.Bass:
    """Build the single-core Bass program (SPMD across cores via in_maps)."""
    mm_dt = {"f32": F32, "f32r": mybir.dt.float32r, "bf16": F32}[mm_dtype]

    nc = bacc.Bacc("TRN2", target_bir_lowering=False, debug=False)
    x_d = nc.dram_tensor("x", [T, E], F32, kind="ExternalInput").ap()
    wq_d = nc.dram_tensor("Wq", [E, H], F32, kind="ExternalInput").ap()
    wk_d = nc.dram_tensor("Wk", [E, H], F32, kind="ExternalInput").ap()
    wv_d = nc.dram_tensor("Wv", [E, H], F32, kind="ExternalInput").ap()
    out_d = nc.dram_tensor("out", [T, H], F32, kind="ExternalOutput").ap()

    with tile.TileContext(nc) as tc:
        with (
            tc.tile_pool(name="const", bufs=1) as const,
            tc.tile_pool(name="xin", bufs=6) as xin,
            tc.tile_pool(name="xt", bufs=NE) as xtp,
            tc.tile_pool(name="proj", bufs=1) as projp,
            tc.tile_pool(name="vaug", bufs=1) as vaugp,
            tc.tile_pool(name="expt", bufs=10) as exptp,
            tc.tile_pool(name="outs", bufs=4) as outsp,
            tc.tile_pool(name="ps_sc", bufs=2, space="PSUM") as ps_sc_p,
            tc.tile_pool(name="ps_tr", bufs=2, space="PSUM") as ps_tr_p,
            tc.tile_pool(name="ps_pm", bufs=2, space="PSUM") as ps_pm_p,
        ):
            # --- constants ---------------------------------------------------
            ident = const.tile([P, P], F32)
            make_identity(nc, ident)
            # Additive causal mask, applied to score PSUM before exp.
            # bigmask[s, u] = -1e30 where u < 384 + s else 0.  For a diagonal
            # j-block at position rel (j = 4g+rel), the slice
            # bigmask[:, 384-rel*128 : 384-rel*128+(rel+1)*128] masks the
            # below-diagonal t-chunks AND the in-block upper triangle.
            bigmask = const.tile([P, GW], F32)
            nc.gpsimd.memset(bigmask, 0.0)
            nc.gpsimd.affine_select(
                out=bigmask, in_=bigmask,
                compare_op=mybir.AluOpType.is_ge,
                fill=-1e30, base=-384,
                pattern=[[1, GW]], channel_multiplier=-1,
            )
            # weights, e-major: [p, c, h] with e = c*128 + p  (SWDGE queue so
            # the x-tile loads own the HWDGE queue from t=0).  Wq and Wk are
            # packed side by side so one M=128 matmul computes both heads'
            # projections: psum rows 0:64 = qT, rows 64:128 = kT.
            wqk_f = const.tile([P, NE, 2 * H], F32, tag="wqkf")
            nc.gpsimd.dma_start(
                out=wqk_f[:, :, :H], in_=wq_d.rearrange("(c p) h -> p c h", p=P))
            nc.gpsimd.dma_start(
                out=wqk_f[:, :, H:], in_=wk_d.rearrange("(c p) h -> p c h", p=P))
            wv_f = const.tile([P, NE, H], F32, tag="wvf")
            nc.gpsimd.dma_start(
                out=wv_f, in_=wv_d.rearrange("(c p) h -> p c h", p=P))
            wqk = const.tile([P, NE, 2 * H], mm_dt, tag="wqk")
            nc.vector.tensor_copy(wqk, wqk_f)
            wv = const.tile([P, NE, H], mm_dt, tag="wv")
            nc.vector.tensor_copy(wv, wv_f)
            ones = const.tile([P, NT, 1], F32, tag="ones")
            nc.gpsimd.memset(ones, 1.0)

            # persistent per-iteration state (allocated fresh each repeat)
            def body(_iv=None):
                xT = [xtp.tile([P, T], mm_dt, tag="xt", name=f"xT{c}") for c in range(NE)]
                qT = projp.tile([H, T], mm_dt, tag="qt")
                kT = projp.tile([H, T], mm_dt, tag="kt")
                vT = projp.tile([H, T], F32, tag="vt")
                # vaug[s, j, :] = [v | 1] per s-chunk j; ones column via DVE
                # copy (f32r memset fails the walrus ISA check)
                vaug = vaugp.tile([P, NT, H + 1], mm_dt, tag="vaug")
                nc.vector.tensor_copy(vaug[:, :, H:H + 1], ones)

                # one-time wait absorber: PE picks up the Pool-engine sem
                # (identity/mask constants) here, minimizing multi-wait
                # EventSemaphore splits on later matmuls
                dmy = ps_tr_p.tile([1, P], F32, tag="tr", name="dmy0")
                nc.tensor.transpose(dmy, ident[:, :1], ident)

                def loads(g):
                    xts = [xin.tile([P, E], F32, tag="xin", name=f"xin{i}")
                           for i in range(CPG)]
                    for q in range(4):
                        lo, hi = q * E // 4, (q + 1) * E // 4
                        for i in range(CPG):
                            r0 = (g * CPG + i) * P
                            nc.sync.dma_start(
                                out=xts[i][:, lo:hi], in_=x_d[r0:r0 + P, lo:hi])
                    return xts

                def tp_units(g, xts):
                    """x-transpose + projections for group g, as a generator
                    of small emission units for software pipelining."""
                    g0 = g * GW

                    def emit_trb(c):
                        ps = ps_tr_p.tile([P, GW], F32, tag="tr", name=f"trb{c}")
                        for ii in range(CPG):
                            nc.tensor.transpose(
                                ps[:, ii * P:(ii + 1) * P],
                                xts[ii][:, c * P:(c + 1) * P], ident)
                        if c >= NE - 2:
                            nc.scalar.copy(xT[c][:, g0:g0 + GW], ps)
                        else:
                            nc.vector.tensor_copy(xT[c][:, g0:g0 + GW], ps)

                    emit_trb(0)
                    yield
                    emit_trb(1)
                    yield
                    psqk = ps_pm_p.tile([P, GW], F32, tag="pm", name="psqk")
                    for c in range(NE):
                        if c + 2 < NE:
                            emit_trb(c + 2)
                        nc.tensor.matmul(
                            psqk, wqk[:, c, :], xT[c][:, g0:g0 + GW],
                            start=(c == 0), stop=(c == NE - 1))
                        yield
                    nc.vector.tensor_copy(qT[:, g0:g0 + GW], psqk[:H, :])
                    # kT: psum partitions 64:128 -> bounce (same partitions)
                    # -> SBUF-to-SBUF DMA down to base-0 partitions
                    ktmp = xin.tile([P, GW], mm_dt, tag="ktmp")
                    nc.vector.tensor_copy(ktmp[H:, :], psqk[H:, :])
                    nc.sync.dma_start(out=kT[:, g0:g0 + GW], in_=ktmp[H:, :])
                    yield
                    psp = ps_pm_p.tile([H, GW], F32, tag="pm", name="psp")
                    for c in range(NE):
                        nc.tensor.matmul(
                            psp, wv[:, c, :], xT[c][:, g0:g0 + GW],
                            start=(c == 0), stop=(c == NE - 1))
                        if c % 2:
                            yield
                    nc.vector.tensor_copy(vT[:, g0:g0 + GW], psp)
                    yield
                    # vaug[:, j, :64] = v rows for this group's s-chunks
                    psv = ps_tr_p.tile([P, CPG, H], F32, tag="tr", name="psv")
                    for ii in range(CPG):
                        nc.tensor.transpose(
                            psv[:, ii, :],
                            vT[:, (g * CPG + ii) * P:(g * CPG + ii + 1) * P],
                            ident[:H, :H])
                    nc.vector.tensor_copy(
                        vaug[:, g * CPG:(g + 1) * CPG, :H], psv)
                    # absorber: pick up the vaug-copy DVE sem on PE early
                    dmyg = ps_tr_p.tile([1, P], F32, tag="tr", name=f"dmy{g}")
                    nc.tensor.transpose(
                        dmyg, vaug[:, g * CPG, :1].bitcast(F32), ident)
                    yield

                def attn_units(g):
                    """scores -> exp -> AV -> normalize for group g."""
                    g0 = g * GW
                    ps_av = ps_pm_p.tile([H + 1, GW], F32, tag="pm", name="ps_av")
                    njb = CPG * (g + 1)          # j-blocks 0 .. 4g+3
                    ets = []

                    def emit_av(m):
                        et_m = ets[m]
                        for hf in range(2):
                            j = 2 * m + hf
                            rel = max(j - CPG * g, 0)
                            nc.tensor.matmul(
                                ps_av[:, rel * P:],
                                vaug[:, j, :],
                                et_m[:, hf * GW + rel * P:(hf + 1) * GW],
                                start=(j == 0), stop=(j == njb - 1))

                    for m in range(njb // 2):
                        ps_s = ps_sc_p.tile([P, 2 * GW], F32, tag="sc")
                        for hf in range(2):
                            j = 2 * m + hf
                            nc.tensor.matmul(
                                ps_s[:, hf * GW:(hf + 1) * GW],
                                kT[:, j * P:(j + 1) * P],
                                qT[:, g0:g0 + GW],
                                start=True, stop=True)
                            rel = j - CPG * g
                            if rel >= 0:
                                # diagonal block: mask only the triangle; the
                                # below-diagonal columns are never read (the
                                # AV matmul is narrowed past them)
                                reg = ps_s[:, hf * GW + rel * P:hf * GW + (rel + 1) * P]
                                nc.vector.tensor_add(
                                    reg, reg, bigmask[:, 384:384 + P])
                        et = exptp.tile([P, 2 * GW], mm_dt, tag="expt")
                        nc.scalar.activation(
                            et, ps_s, mybir.ActivationFunctionType.Exp,
                            scale=float(H) ** -0.5)
                        ets.append(et)
                        yield
                        if m >= 1:
                            emit_av(m - 1)
                            yield
                    emit_av(njb // 2 - 1)
                    yield

                    # normalize + write out
                    avT = outsp.tile([H + 1, GW], F32, tag="avt")
                    nc.scalar.copy(avT, ps_av)
                    for ii in range(CPG):
                        i = g * CPG + ii
                        ps_o = ps_tr_p.tile([P, H + 1], F32, tag="tr", name="ps_o")
                        nc.tensor.transpose(
                            ps_o, avT[:, ii * P:(ii + 1) * P],
                            ident[:H + 1, :H + 1])
                        rcp = outsp.tile([P, 1], F32, tag="rcp")
                        nc.vector.reciprocal(rcp, ps_o[:, H:H + 1])
                        ot = outsp.tile([P, H], F32, tag="ot")
                        nc.vector.tensor_scalar_mul(ot, ps_o[:, :H], rcp)
                        nc.gpsimd.dma_start(
                            out=out_d[i * P:(i + 1) * P, :], in_=ot)
                        yield

                # software pipeline: attention(g) interleaves with
                # loads + transposes + projections of group g+1
                done = object()
                gen_tp = tp_units(0, loads(0))
                for _ in gen_tp:
                    pass
                for g in range(NG):
                    gen_att = attn_units(g)
                    gen_tp = tp_units(g + 1, loads(g + 1)) if g + 1 < NG else None
                    while True:
                        a = next(gen_att, done)
                        t = next(gen_tp, done) if gen_tp is not None else done
                        if a is done and t is done:
                            break

            if repeat == 1:
                body()
            else:
                tc.For_i_unrolled(0, repeat, 1, body, max_unroll=1)

    nc.compile()
    return nc


class _Runner:
    """Cached jitted SPMD executor for one built nc.

    run_bass_kernel_spmd rebuilds jax.jit(shard_map(...)) on every call,
    which forces a full XLA retrace + NEFF reload each time.  Building the
    jitted callable once (and keeping inputs device-resident) turns repeat
    calls from ~1.4 s into milliseconds, which the timing harness needs.
    """

    def __init__(self, nc):
        import jax
        from jax.experimental.shard_map import shard_map
        from jax.sharding import Mesh, NamedSharding, PartitionSpec
        from concourse import bass2jax, mybir as mb

        bass2jax.install_neuronx_cc_hook()
        in_names, out_names, out_avals = [], [], []
        for alloc in nc.m.functions[0].allocations:
            if not isinstance(alloc, mb.MemoryLocationSet):
                continue
            name = alloc.memorylocations[0].name
            if alloc.kind == "ExternalInput":
                in_names.append(name)
            elif alloc.kind == "ExternalOutput":
                out_names.append(name)
                out_avals.append(jax.core.ShapedArray(
                    tuple(alloc.tensor_shape), mb.dt.np(alloc.dtype)))
        assert nc.dbg_addr is None
        part_name = nc.partition_id_tensor.name if nc.partition_id_tensor else None
        if part_name is not None:
            in_names = [n for n in in_names if n != part_name]
        self.in_names, self.out_names, self.out_avals = in_names, out_names, out_avals
        n_params = len(in_names)
        all_names = in_names + out_names
        if part_name is not None:
            all_names = all_names + [part_name]

        def _body(*args):
            operands = list(args)
            if part_name is not None:
                operands.append(bass2jax.partition_id_tensor())
            outs = bass2jax._bass_exec_p.bind(
                *operands,
                out_avals=tuple(out_avals),
                in_names=tuple(all_names),
                out_names=tuple(out_names),
                lowering_input_output_aliases=(),
                sim_require_finite=True,
                sim_require_nnan=True,
                nc=nc,
            )
            return tuple(outs)

        devices = jax.devices()[:B]
        self.mesh = Mesh(np.asarray(devices), ("core",))
        self.spec = PartitionSpec("core")
        self.sharding = NamedSharding(self.mesh, self.spec)
        nin = n_params + len(out_names)
        self.fn = jax.jit(
            shard_map(
                _body, mesh=self.mesh,
                in_specs=(self.spec,) * nin,
                out_specs=(self.spec,) * len(out_names),
                check_rep=False,
            ),
            donate_argnums=tuple(range(n_params, nin)),
            keep_unused=True,
        )
        self._dev_inputs = {}

    def prep_inputs(self, in_maps, cache_key=None):
        """Concat per-core inputs to global arrays, optionally device-cached."""
        import jax
        if cache_key is not None and cache_key in self._dev_inputs:
            return self._dev_inputs[cache_key]
        concat = [
            np.concatenate([np.asarray(m[n]) for m in in_maps], axis=0)
            for n in self.in_names
        ]
        arrs = [jax.device_put(a, self.sharding) for a in concat]
        jax.block_until_ready(arrs)
        if cache_key is not None:
            self._dev_inputs[cache_key] = arrs
        return arrs

    def __call__(self, dev_inputs, block=True):
        import jax
        zeros = [
            np.zeros((B * av.shape[0], *av.shape[1:]), av.dtype)
            for av in self.out_avals
        ]
        outs = self.fn(*dev_inputs, *zeros)
        if block:
            jax.block_until_ready(outs)
        return outs

    def gather(self, outs):
        o = np.asarray(outs[0])
        return o.reshape(B, -1, o.shape[-1])


def _get_runner(mm_dtype: str, repeat: int) -> "_Runner":
    key = (mm_dtype, repeat)
    if key not in _NC_CACHE:
        _NC_CACHE[key] = _Runner(build_attention_nc(mm_dtype, repeat))
    return _NC_CACHE[key]


def _make_in_maps(inputs: dict):
    x = np.asarray(inputs["x"], dtype=np.float32)
    wq = np.ascontiguousarray(np.asarray(inputs["Wq"], dtype=np.float32))
    wk = np.ascontiguousarray(np.asarray(inputs["Wk"], dtype=np.float32))
    wv = np.ascontiguousarray(np.asarray(inputs["Wv"], dtype=np.float32))
    return [
        {"x": np.ascontiguousarray(x[i]), "Wq": wq, "Wk": wk, "Wv": wv}
        for i in range(B)
    ]


def run_spmd(inputs: dict, mm_dtype: str = MM_DTYPE, repeat: int = 1,
             cache_key=None):
    r = _get_runner(mm_dtype, repeat)
    dev = r.prep_inputs(_make_in_maps(inputs), cache_key=cache_key)
    return r.gather(r(dev))


def kernel(**inputs) -> np.ndarray:
    return run_spmd(inputs, MM_DTYPE, repeat=1)
